# revision 27
# baseline (speedup 1.0000x reference)
"""Binary-cross-entropy custom loss on 8 Trainium2 NeuronCores.

reference math:
    ll   = lab*log_sigmoid(p) + (1-lab)*log_sigmoid(-p) = lab*p - softplus(p)
    loss = -sum(ll) / ((1 + neg) * pos),  pos = sum(lab), neg = N - pos

Data-parallel over N=2^24, 2M elements per core.  Per-core engine split:
  ACT : e = exp(p); softplus = ln(e + 1) with accum_out -> per-partition sums
        (this build has no softplus ACT table; exp/ln share one table set,
        manually preloaded so the insertion pass emits no per-tile reloads)
  DVE : prod = lab * p (bf16 out, one pass) + per-tile pos counts
  PE  : ones-vector matmuls accumulate sum(lab*p) into PSUM
  host: float64 scalar combine of the 8 cores' partials

Inputs are packed host-side into one [P, 16384] f32 tensor per core: for
each tile, Fi/2 f32 lanes of p as fp16 followed by Fi/2 lanes of labels
as fp16 (lossless 0/1).  One dma_start per tile (single semaphore -- the
CoreV3 ISA has one sync-wait slot per instruction).  fp16 p quantization
adds ~1e-6 relative error to the loss (sums of ~16M near-random-sign
rounding errors) while halving DMA traffic and enabling the DVE 2x 16-bit
mode.  Tile sizes ramp up/down (small first tiles so compute starts
sooner, small last tile so the tail is not gated by a 3 MB transfer).
"""
import sys

if "/opt/trn_rl_repo" not in sys.path:
    sys.path.insert(0, "/opt/trn_rl_repo")

import ml_dtypes
import numpy as np

import concourse.bacc as bacc
import concourse.bass as bass
import concourse.mybir as mybir
import concourse.tile as tile
from concourse.bass_utils import run_bass_kernel_spmd
from concourse.hw_specs import get_activation_tables

N = 16777216
N_CORES = 8
P = 128
TILES = [1024, 2048, 4096, 4096, 4096, 1024]  # per-tile free-dim Fi
assert sum(TILES) * P * N_CORES == N
MM = 512  # matmul free-dim chunk (one PSUM bank)
TOTALC = sum(TILES)  # f32 lanes per partition row (bf16 p + bf16 lab)

_NC_CACHE = None


def build_nc(tiles=None):
    """Build the (single-program, 8-core SPMD) Bass module."""
    tiles = TILES if tiles is None else tiles
    totalc = sum(tiles)
    T = len(tiles)
    nc = bacc.Bacc(
        "TRN2",
        target_bir_lowering=False,
        debug=False,
        enable_asserts=False,
        num_devices=N_CORES,
    )
    data_dram = nc.dram_tensor("data", [P, totalc], mybir.dt.float32, kind="ExternalInput").ap()
    out_dram = nc.dram_tensor("partials", [P, 3], mybir.dt.float32, kind="ExternalOutput").ap()

    with tile.TileContext(nc) as tc:
        # Preload the one ACT table set containing BOTH exp and ln; the
        # auto-insertion pass then sees every activation's table resident.
        act_tables = list(get_activation_tables(nc.m.arch).keys())
        nle_id = act_tables.index("natural_log_exp_and_others")
        nc.scalar.add_instruction(mybir.InstLoadActFuncSet(
            name=nc.get_next_instruction_name(), ins=[], outs=[],
            act_func_set_id=nle_id,
        ))
        with tc.tile_pool(name="io", bufs=4) as io_pool, \
             tc.tile_pool(name="ajunk", bufs=2) as act_junk, \
             tc.tile_pool(name="vjunk", bufs=2) as dve_junk, \
             tc.tile_pool(name="psum", bufs=1, space="PSUM") as psum_pool, \
             tc.tile_pool(name="acc", bufs=1) as acc_pool:
            sp_cols = acc_pool.tile([P, T], mybir.dt.float32)
            pos_cols = acc_pool.tile([P, T], mybir.dt.float32)
            sums = acc_pool.tile([P, 3], mybir.dt.float32)
            ones_bf = acc_pool.tile([P, 1], mybir.dt.float16)
            nc.gpsimd.memset(ones_bf[:], 1.0)
            nc.gpsimd.memset(sums[:], 0.0)
            psum_lp = psum_pool.tile([1, MM], mybir.dt.float32)
            fmax = max(tiles)
            n_mms = sum(f // MM for f in tiles)
            c0 = 0
            mm_idx = 0
            for i, F in enumerate(tiles):
                w = F
                data_t = io_pool.tile([P, fmax], mybir.dt.float32,
                                      name="data_t")
                nc.sync.dma_start(data_t[:, 0:w], data_dram[:, c0:c0 + w])
                p_t = data_t[:, 0:F // 2].bitcast(mybir.dt.float16)  # [P, F]
                lab_bf = data_t[:, F // 2:w].bitcast(mybir.dt.float16)  # [P, F]

                e_t = act_junk.tile([P, fmax], mybir.dt.float16, name="e_t")
                nc.scalar.activation(e_t[:, 0:F], p_t, mybir.ActivationFunctionType.Exp)
                sp_junk = act_junk.tile([P, fmax], mybir.dt.float32, name="sp_junk")
                nc.scalar.activation(
                    sp_junk[:, 0:F],
                    e_t[:, 0:F],
                    mybir.ActivationFunctionType.Ln,
                    bias=1.0,
                    accum_out=sp_cols[:, i:i + 1],
                )
                prod_bf = dve_junk.tile([P, fmax], mybir.dt.float16, name="prod_bf")
                nc.vector.tensor_mul(prod_bf[:, 0:F], lab_bf, p_t)
                nc.vector.tensor_reduce(
                    out=pos_cols[:, i:i + 1],
                    in_=lab_bf,
                    op=mybir.AluOpType.add,
                    axis=mybir.AxisListType.X,
                )
                for j in range(F // MM):
                    nc.tensor.matmul(
                        psum_lp[:],
                        ones_bf[:],
                        prod_bf[:, j * MM:(j + 1) * MM],
                        start=mm_idx == 0,
                        stop=mm_idx == n_mms - 1,
                        skip_group_check=True,
                    )
                    mm_idx += 1
                c0 += w
            # Tail: per-partition softplus sums -> col 0; scalar lab*p sum
            # (partition 0 only) -> col 1; per-partition lab counts -> col 2.
            nc.vector.reduce_sum(out=sums[:, 0:1], in_=sp_cols[:], axis=mybir.AxisListType.X)
            nc.vector.reduce_sum(out=sums[0:1, 1:2], in_=psum_lp[:], axis=mybir.AxisListType.X)
            nc.vector.reduce_sum(out=sums[:, 2:3], in_=pos_cols[:], axis=mybir.AxisListType.X)
            nc.sync.dma_start(out_dram[:], sums[:])
    nc.compile()  # bacc legalization: split multi-waits via event semaphores
    return nc


def get_nc():
    global _NC_CACHE
    if _NC_CACHE is None:
        _NC_CACHE = build_nc()
    return _NC_CACHE


def pack_inputs(pv, lb, tiles):
    """pv, lb: [cores, elems] -> packed bf16-pair [cores, P, totalc] f32."""
    n_cores = pv.shape[0]
    totalc = sum(tiles)
    data = np.empty((n_cores, P, totalc), dtype=np.float32)
    e0 = 0
    c0 = 0
    for F in tiles:
        ne = P * F
        data[:, :, c0:c0 + F // 2] = (
            pv[:, e0:e0 + ne].reshape(n_cores, P, F)
            .astype(np.float16).view(np.float32)
        )
        data[:, :, c0 + F // 2:c0 + F] = (
            lb[:, e0:e0 + ne].reshape(n_cores, P, F)
            .astype(np.float16).view(np.float32)
        )
        e0 += ne
        c0 += F
    return data


def shard_inputs(predicted_values, labels):
    pv = np.ascontiguousarray(predicted_values, dtype=np.float32).reshape(N_CORES, -1)
    lb = np.ascontiguousarray(labels, dtype=np.int32).reshape(N_CORES, -1)
    data = pack_inputs(pv, lb, TILES)
    return [{"data": data[c]} for c in range(N_CORES)]


def combine(results):
    """results: list of 8 dicts with 'partials' [128,3] -> loss [1] f32.

    col 0: per-partition softplus sums; col 1 row 0: sum(lab*p);
    col 2: per-partition lab counts."""
    s_sp = s_lp = pos = 0.0
    for r in results:
        part = r["partials"].astype(np.float64)
        s_sp += part[:, 0].sum()
        s_lp += part[0, 1]
        pos += part[:, 2].sum()
    neg = float(N) - pos
    loss = (s_sp - s_lp) / ((1.0 + neg) * pos)
    return np.array([loss], dtype=np.float32)


def kernel(predicted_values, labels):
    assert predicted_values.shape == (N,) and labels.shape == (N,)
    in_maps = shard_inputs(predicted_values, labels)
    res = run_bass_kernel_spmd(get_nc(), in_maps, core_ids=list(range(N_CORES)))
    return combine(res.results)


if __name__ == "__main__":
    rng = np.random.default_rng(0)
    pv = rng.standard_normal(N).astype(np.float32)
    lb = rng.integers(0, 2, size=N).astype(np.int32)
    out = kernel(pv, lb)
    print("loss:", out)
